# revision 1
# baseline (speedup 1.0000x reference)
"""DeepGCN (GENConv softmax-aggregation, 4 layers) on 8 Trainium2 NeuronCores.

Strategy (graph/data parallel per sharding hint):
  - Nodes partitioned contiguously across 8 cores (6250 each); edges assigned
    to the core owning their dst node, sorted by (dst tile, src), padded per
    128-node tile so every core runs an identical (SPMD) program.
  - Per layer: source rows are gathered from a replicated node-major bf16
    [50000,128] DRAM table via SWDGE indirect DMA (one 128-edge tile per
    instruction, 256B/row descriptors; src-sorted slots for HBM locality).
    SWDGE descriptor generation on the Pool engine (~1.4us per 128-edge
    tile) is the hardware floor of this kernel; hardware- and
    software-dynamic DMA serialize on TRN2, so no second indirect path can
    run concurrently.
  - The per-(edge,node-slot) aggregation indicator is static across layers:
    precomputed on host as fp8e4 and streamed from DRAM; aggregation runs as
    fp8 x bf16 indicator matmuls accumulating [denom | num] in PSUM per
    128-node tile.
  - Edge chain (aw=attr*w, +gather, relu, exp, msg*ez) runs bf16 group-wide
    on DVE + Act (16-bit DVE fast modes); per-node MLP is bf16 on PE;
    residual h stays f32 in SBUF; transposes run as PE matmuls with the skip
    connection / edge bias accumulated into the same PSUM.
  - Between layers each core's slice of r'=relu(BN(h))+edge_b is AllGathered
    (bf16, Shared scratchpad) into the next layer's gather table.
  - Graph mean-pool partials ([64,128] per core) are summed on host; the tiny
    136x2 classifier runs on host.
"""

import numpy as np
import ml_dtypes

import concourse.bass as bass
import concourse.bacc as bacc
import concourse.tile as tile
from concourse import mybir
from concourse.masks import make_identity
from concourse.bass_utils import run_bass_kernel_spmd

F32 = mybir.dt.float32
BF16 = mybir.dt.bfloat16
I32 = mybir.dt.int32
FP8 = mybir.dt.float8e4

NP_BF16 = ml_dtypes.bfloat16
NP_FP8 = ml_dtypes.float8_e4m3

N, E, C, H, L, G, K, NCLS = 50000, 500000, 256, 128, 4, 64, 8, 2
NCORES = 8
NPC = N // NCORES          # 6250 nodes per core
NT = (NPC + 127) // 128    # 49 node tiles per core
NPC_PAD = NT * 128         # 6272
Q = 16                     # edge tiles per chain group
EPS_BN = 1e-5
P = 128

_cache = {}


def _ap_view(t, extra_offset, pattern):
    base = t[:]
    return bass.AP(base.tensor, base.offset + extra_offset, [base.ap[0]] + pattern)


def _build(ET, t_vals):
    TE = int(np.sum(ET))
    nt_of = np.repeat(np.arange(NT), ET)
    tile_starts = np.concatenate([[0], np.cumsum(ET)])
    first_of = set(tile_starts[:-1].tolist())
    last_of = set((tile_starts[1:] - 1).tolist())
    NG = (TE + Q - 1) // Q

    nc = bacc.Bacc("TRN2", target_bir_lowering=False, debug=False,
                   num_devices=NCORES)

    # ---- kernel I/O ----
    xT_in = nc.dram_tensor("xT", [C, NPC_PAD], BF16, kind="ExternalInput")
    esrc_in = nc.dram_tensor("esrc", [P, TE], I32, kind="ExternalInput")
    eattr_in = nc.dram_tensor("eattr", [P, TE], BF16, kind="ExternalInput")
    ind_in = nc.dram_tensor("ind8", [P, TE * P], FP8, kind="ExternalInput")
    batch_in = nc.dram_tensor("batch", [P, NT], I32, kind="ExternalInput")
    bcast_in = nc.dram_tensor("bcast", [2 * L, P, P], BF16, kind="ExternalInput")
    lsw_in = nc.dram_tensor("lsw", [C, H], BF16, kind="ExternalInput")
    ldw_in = nc.dram_tensor("ldw", [C, H], BF16, kind="ExternalInput")
    ldb_in = nc.dram_tensor("ldb", [H], F32, kind="ExternalInput")
    w1_in = nc.dram_tensor("w1f", [L, H, 2 * H], BF16, kind="ExternalInput")
    b1_in = nc.dram_tensor("b1f", [L, 2 * H], F32, kind="ExternalInput")
    w2_in = nc.dram_tensor("w2", [L, 2 * H, H], BF16, kind="ExternalInput")
    b2_in = nc.dram_tensor("b2", [L, H], F32, kind="ExternalInput")
    bns_in = nc.dram_tensor("bns", [L, H], F32, kind="ExternalInput")
    bnb_in = nc.dram_tensor("bnb", [L, H], F32, kind="ExternalInput")
    pooled_out = nc.dram_tensor("pooled", [G, H], F32, kind="ExternalOutput")

    with tile.TileContext(nc) as tc:
        with (
            tc.tile_pool(name="persist", bufs=1) as pp,
            tc.tile_pool(name="wl", bufs=1) as wl,
            tc.tile_pool(name="edge", bufs=2) as ep,
            tc.tile_pool(name="node", bufs=4) as npool,
            tc.tile_pool(name="psA", bufs=3, space="PSUM") as psA,
            tc.tile_pool(name="psB", bufs=1, space="PSUM") as psB,
            tc.tile_pool(name="psC", bufs=1, space="PSUM") as psC,
            tc.tile_pool(name="psT", bufs=2, space="PSUM") as psT,
            tc.tile_pool(name="psP", bufs=1, space="PSUM") as psP,
            tc.tile_pool(name="dram", bufs=4, space="DRAM") as dp,
        ):
            # ---------- persistent state ----------
            hT = pp.tile([P, NPC_PAD], F32, tag="hT")        # residual [H, nodes]
            skipT = pp.tile([P, NPC_PAD], BF16, tag="skipT")  # r_l skip [H, nodes]

            ident = pp.tile([P, P], BF16, tag="ident")
            make_identity(nc, ident[:])

            idx_all = pp.tile([P, TE], I32, tag="idx")
            nc.sync.dma_start(idx_all[:], esrc_in[:])
            attr_s = pp.tile([P, TE], BF16, tag="attrs")
            nc.sync.dma_start(attr_s[:], eattr_in[:])
            batch_i = pp.tile([P, NT], I32, tag="batchi")
            nc.sync.dma_start(batch_i[:], batch_in[:])
            batch_f = pp.tile([P, NT], F32, tag="batchf")
            nc.vector.tensor_copy(out=batch_f[:], in_=batch_i[:])

            iota_ig = pp.tile([P, G], I32, tag="iotaig")
            nc.gpsimd.iota(iota_ig[:], pattern=[[1, G]], base=0,
                           channel_multiplier=0)
            iota_g = pp.tile([P, G], F32, tag="iotag")
            nc.vector.tensor_copy(out=iota_g[:], in_=iota_ig[:])

            # broadcast tiles: [srcb, wbc0..3, ebbc1..3]
            srcb_bc = pp.tile([P, P], BF16, tag="srcbbc")
            nc.sync.dma_start(srcb_bc[:], bcast_in[0])
            wbc = []
            for l in range(L):
                wb = wl.tile([P, P], BF16, tag=f"wbc{l}")
                nc.sync.dma_start(wb[:], bcast_in[1 + l])
                wbc.append(wb)
            ebbc = {}
            for l in range(1, L):
                eb = wl.tile([P, P], BF16, tag=f"ebbc{l}")
                nc.sync.dma_start(eb[:], bcast_in[4 + l])
                ebbc[l] = eb

            # projection weights
            lsw0 = pp.tile([P, H], BF16, tag="lsw0")
            lsw1 = pp.tile([P, H], BF16, tag="lsw1")
            ldw0 = pp.tile([P, H], BF16, tag="ldw0")
            ldw1 = pp.tile([P, H], BF16, tag="ldw1")
            nc.sync.dma_start(lsw0[:], lsw_in[0:P, :])
            nc.sync.dma_start(lsw1[:], lsw_in[P : 2 * P, :])
            nc.sync.dma_start(ldw0[:], ldw_in[0:P, :])
            nc.sync.dma_start(ldw1[:], ldw_in[P : 2 * P, :])
            ldb_v = pp.tile([P, 1], F32, tag="ldbv")
            nc.sync.dma_start(ldb_v[:], ldb_in[:, None])

            # per-layer MLP / norm params
            w1s, b1a, b1b, w2a, w2b, b2v, bnsv, bnbv = [], [], [], [], [], [], [], []
            for l in range(L):
                w1 = wl.tile([P, 2 * H], BF16, tag=f"w1{l}")
                nc.sync.dma_start(w1[:], w1_in[l])
                w1s.append(w1)
                ba = wl.tile([P, 1], F32, tag=f"b1a{l}")
                nc.sync.dma_start(ba[:], b1_in[l, 0:H][:, None])
                b1a.append(ba)
                bb = wl.tile([P, 1], F32, tag=f"b1b{l}")
                nc.sync.dma_start(bb[:], b1_in[l, H : 2 * H][:, None])
                b1b.append(bb)
                wa = wl.tile([P, H], BF16, tag=f"w2a{l}")
                nc.sync.dma_start(wa[:], w2_in[l, 0:H, :])
                w2a.append(wa)
                wb2 = wl.tile([P, H], BF16, tag=f"w2b{l}")
                nc.sync.dma_start(wb2[:], w2_in[l, H : 2 * H, :])
                w2b.append(wb2)
                bv = wl.tile([P, 1], F32, tag=f"b2{l}")
                nc.sync.dma_start(bv[:], b2_in[l, :][:, None])
                b2v.append(bv)
                sv = wl.tile([P, 1], F32, tag=f"bns{l}")
                nc.sync.dma_start(sv[:], bns_in[l, :][:, None])
                bnsv.append(sv)
                bvv = wl.tile([P, 1], F32, tag=f"bnb{l}")
                nc.sync.dma_start(bvv[:], bnb_in[l, :][:, None])
                bnbv.append(bvv)

            # gather tables (DRAM, node-major bf16)
            g_local = [dp.tile([NPC, H], BF16, tag="glocal", name=f"glocal{i}")
                       for i in range(L)]
            g_full = [dp.tile([N, H], BF16, tag="gfull", name=f"gfull{i}",
                              addr_space="Shared")
                      for i in range(L)]

            def all_gather(l):
                nc.gpsimd.collective_compute(
                    "AllGather", mybir.AluOpType.bypass,
                    replica_groups=[list(range(NCORES))],
                    ins=[g_local[l].opt()], outs=[g_full[l].opt()],
                )

            # ---------- phase A: layer-0 projections ----------
            XCH = 4  # node tiles per x chunk load
            for c0 in range(0, NT, XCH):
                cn = min(XCH, NT - c0)
                nb0 = c0 * 128
                xc0 = npool.tile([P, XCH * P], BF16, tag="xc0")
                xc1 = npool.tile([P, XCH * P], BF16, tag="xc1")
                nc.sync.dma_start(xc0[:, 0 : cn * 128],
                                  xT_in[0:P, nb0 : nb0 + cn * 128])
                nc.sync.dma_start(xc1[:, 0 : cn * 128],
                                  xT_in[P : 2 * P, nb0 : nb0 + cn * 128])
                for ci in range(cn):
                    nt = c0 + ci
                    nb = nt * 128
                    rows = min(128, NPC - nb)
                    xT0 = xc0[:, ci * 128 : (ci + 1) * 128]
                    xT1 = xc1[:, ci * 128 : (ci + 1) * 128]

                    ps_xs = psB.tile([P, 2 * H], F32, space="PSUM", tag="mlp1")
                    nc.tensor.matmul(out=ps_xs[:, 0:H], lhsT=xT0, rhs=lsw0[:],
                                     start=True, stop=False)
                    nc.tensor.matmul(out=ps_xs[:, 0:H], lhsT=xT1, rhs=lsw1[:],
                                     start=False, stop=True)
                    rw = npool.tile([P, H], BF16, tag="rw")
                    nc.vector.tensor_add(out=rw[:], in0=ps_xs[:, 0:H],
                                         in1=srcb_bc[:, 0:H])
                    nc.sync.dma_start(g_local[0][nb : nb + rows, :],
                                      rw[:rows, :])

                    ps_xd = psC.tile([P, H], F32, space="PSUM", tag="mlp2")
                    nc.tensor.matmul(out=ps_xd[:], lhsT=ldw0[:], rhs=xT0,
                                     start=True, stop=False)
                    nc.tensor.matmul(out=ps_xd[:], lhsT=ldw1[:], rhs=xT1,
                                     start=False, stop=True)
                    nc.scalar.activation(
                        out=skipT[:, nb : nb + 128], in_=ps_xd[:],
                        func=mybir.ActivationFunctionType.Identity,
                        bias=ldb_v[:, :1], scale=1.0)

            all_gather(0)

            # ---------- layers ----------
            pool_ps = None
            for l in range(L):
                ps_agg = {}
                for g in range(NG):
                    j0 = g * Q
                    qw = min(Q, TE - j0)
                    W = qw * 128
                    gx = ep.tile([P, Q * 128], BF16, tag="gx", bufs=8)
                    for k in range(qw):
                        nc.gpsimd.indirect_dma_start(
                            out=gx[:, k * 128 : (k + 1) * 128],
                            out_offset=None,
                            in_=g_full[l][:],
                            in_offset=bass.IndirectOffsetOnAxis(
                                ap=idx_all[:, j0 + k : j0 + k + 1], axis=0),
                        )
                    # u = relu(attr*w + gx)
                    aw = ep.tile([P, Q * 128], BF16, tag="aw", bufs=4)
                    av = _ap_view(attr_s, j0, [[1, qw], [0, 128]])
                    wv = _ap_view(wbc[l], 0, [[0, qw], [1, 128]])
                    nc.vector.tensor_tensor(out=aw[:, 0:W], in0=av, in1=wv,
                                            op=mybir.AluOpType.mult)
                    u = ep.tile([P, Q * 128], BF16, tag="u", bufs=6)
                    nc.vector.tensor_add(out=u[:, 0:W], in0=aw[:, 0:W],
                                         in1=gx[:, 0:W])
                    nc.scalar.activation(out=u[:, 0:W], in_=u[:, 0:W],
                                         func=mybir.ActivationFunctionType.Relu,
                                         scale=1.0)
                    # emz interleaved [ez | msg*ez] per edge tile
                    emz = ep.tile([P, Q * 256], BF16, tag="emz", bufs=6)
                    msg_v = _ap_view(u, 0, [[128, qw], [1, 128]])
                    ez_v = _ap_view(emz, 0, [[256, qw], [1, 128]])
                    mez_v = _ap_view(emz, 128, [[256, qw], [1, 128]])
                    nc.scalar.activation(out=ez_v, in_=msg_v,
                                         func=mybir.ActivationFunctionType.Exp,
                                         scale=float(t_vals[l]))
                    nc.vector.tensor_tensor(out=mez_v, in0=msg_v, in1=ez_v,
                                            op=mybir.AluOpType.mult)
                    # static indicator, fp8 from DRAM
                    indt = ep.tile([P, Q * 128], FP8, tag="ind", bufs=6)
                    nc.sync.dma_start(indt[:, 0:W],
                                      ind_in[:, j0 * 128 : j0 * 128 + W])
                    for k in range(qw):
                        j = j0 + k
                        nt = int(nt_of[j])
                        if j in first_of:
                            ps_agg[nt] = psA.tile(
                                [P, 2 * H], F32, space="PSUM", tag="agg",
                                name=f"agg{l}_{nt}", bufs=2)
                        nc.tensor.matmul(
                            out=ps_agg[nt][:],
                            lhsT=indt[:, k * 128 : (k + 1) * 128],
                            rhs=emz[:, k * 256 : (k + 1) * 256],
                            start=(j in first_of), stop=(j in last_of),
                        )
                        if j not in last_of:
                            continue
                        # ---------- node phase for nt ----------
                        nb = nt * 128
                        rows = min(128, NPC - nb)
                        pa = ps_agg.pop(nt)
                        dmax = npool.tile([P, H], F32, tag="dmax")
                        nc.vector.tensor_scalar(out=dmax[:], in0=pa[:, 0:H],
                                                scalar1=1e-16, scalar2=None,
                                                op0=mybir.AluOpType.max)
                        drec = npool.tile([P, H], F32, tag="drec")
                        nc.vector.reciprocal(out=drec[:], in_=dmax[:])
                        aggs = npool.tile([P, H], BF16, tag="aggs")
                        nc.vector.tensor_mul(out=aggs[:], in0=pa[:, H : 2 * H],
                                             in1=drec[:])
                        # outT = aggs^T + skip
                        tp = psT.tile([P, P], F32, space="PSUM", tag="trps")
                        nc.tensor.matmul(out=tp[:], lhsT=aggs[:], rhs=ident[:],
                                         start=True, stop=False)
                        nc.tensor.matmul(out=tp[:], lhsT=ident[:],
                                         rhs=skipT[:, nb : nb + 128],
                                         start=False, stop=True)
                        outT = npool.tile([P, P], BF16, tag="outT")
                        nc.scalar.activation(
                            out=outT[:], in_=tp[:],
                            func=mybir.ActivationFunctionType.Copy)
                        # MLP
                        pm1 = psB.tile([P, 2 * H], F32, space="PSUM", tag="mlp1")
                        nc.tensor.matmul(out=pm1[:, 0:H], lhsT=w1s[l][:, 0:H],
                                         rhs=outT[:], start=True, stop=True)
                        nc.tensor.matmul(out=pm1[:, H : 2 * H],
                                         lhsT=w1s[l][:, H : 2 * H],
                                         rhs=outT[:], start=True, stop=True)
                        h1a = npool.tile([P, P], BF16, tag="h1a")
                        nc.scalar.activation(
                            out=h1a[:], in_=pm1[:, 0:H],
                            func=mybir.ActivationFunctionType.Relu,
                            bias=b1a[l][:, :1], scale=1.0)
                        h1b = npool.tile([P, P], BF16, tag="h1b")
                        nc.scalar.activation(
                            out=h1b[:], in_=pm1[:, H : 2 * H],
                            func=mybir.ActivationFunctionType.Relu,
                            bias=b1b[l][:, :1], scale=1.0)
                        pm2 = psC.tile([P, H], F32, space="PSUM", tag="mlp2")
                        nc.tensor.matmul(out=pm2[:], lhsT=w2a[l][:], rhs=h1a[:],
                                         start=True, stop=False)
                        nc.tensor.matmul(out=pm2[:], lhsT=w2b[l][:], rhs=h1b[:],
                                         start=False, stop=True)
                        hslice = hT[:, nb : nb + 128]
                        if l == 0:
                            b2bc = _ap_view(b2v[l], 0, [[0, 128]])
                            nc.vector.tensor_add(out=hslice, in0=pm2[:],
                                                 in1=b2bc)
                        else:
                            nc.vector.scalar_tensor_tensor(
                                out=hslice, in0=pm2[:], scalar=b2v[l][:, :1],
                                in1=hslice, op0=mybir.AluOpType.add,
                                op1=mybir.AluOpType.add)
                        if l < L - 1:
                            # r_{l+1} = relu(bn_{l+1}(h)); also next skip
                            nc.scalar.activation(
                                out=skipT[:, nb : nb + 128], in_=hslice,
                                func=mybir.ActivationFunctionType.Relu,
                                bias=bnbv[l + 1][:, :1], scale=bnsv[l + 1][:, :1])
                            tp4 = psT.tile([P, P], F32, space="PSUM", tag="trps")
                            nc.tensor.matmul(out=tp4[:],
                                             lhsT=skipT[:, nb : nb + 128],
                                             rhs=ident[:], start=True,
                                             stop=False)
                            nc.tensor.matmul(out=tp4[:], lhsT=ident[:],
                                             rhs=ebbc[l + 1][:],
                                             start=False, stop=True)
                            rw2 = npool.tile([P, H], BF16, tag="rw")
                            nc.scalar.activation(
                                out=rw2[:], in_=tp4[:, 0:H],
                                func=mybir.ActivationFunctionType.Copy)
                            nc.sync.dma_start(
                                g_local[l + 1][nb : nb + rows, :],
                                rw2[:rows, :])
                        else:
                            # final norm (layer 0 params) + pooling partials
                            fT = npool.tile([P, P], BF16, tag="fT")
                            nc.scalar.activation(
                                out=fT[:], in_=hslice,
                                func=mybir.ActivationFunctionType.Relu,
                                bias=bnbv[0][:, :1], scale=bnsv[0][:, :1])
                            tp5 = psT.tile([P, P], F32, space="PSUM",
                                           tag="trps")
                            nc.tensor.matmul(out=tp5[:], lhsT=fT[:],
                                             rhs=ident[:], start=True,
                                             stop=True)
                            fr = npool.tile([P, P], BF16, tag="fr")
                            nc.scalar.activation(
                                out=fr[:], in_=tp5[:],
                                func=mybir.ActivationFunctionType.Copy)
                            gind = npool.tile([P, G], BF16, tag="gind")
                            bv2 = _ap_view(batch_f, nt, [[1, 1], [0, G]])
                            nc.vector.tensor_tensor(out=gind[:], in0=bv2,
                                                    in1=iota_g[:],
                                                    op=mybir.AluOpType.is_equal)
                            if pool_ps is None:
                                pool_ps = psP.tile([G, H], F32, space="PSUM",
                                                   tag="pool")
                            nc.tensor.matmul(out=pool_ps[:], lhsT=gind[:, 0:G],
                                             rhs=fr[:], start=(nt == 0),
                                             stop=(nt == NT - 1))
                if l < L - 1:
                    all_gather(l + 1)

            pool_s = pp.tile([G, H], F32, tag="pools")
            nc.vector.tensor_copy(out=pool_s[:], in_=pool_ps[:])
            nc.sync.dma_start(pooled_out[:], pool_s[:])

    nc.compile()
    return nc


def _prep(edge_index, edge_attr):
    src = edge_index[0].astype(np.int64)
    dst = edge_index[1].astype(np.int64)
    core = dst // NPC
    tloc = (dst % NPC) // 128

    cnt = np.zeros((NCORES, NT), np.int64)
    np.add.at(cnt, (core, tloc), 1)
    ET = np.maximum(np.ceil(cnt.max(axis=0) / 128.0).astype(np.int64), 1)
    TE = int(ET.sum())
    starts = (np.concatenate([[0], np.cumsum(ET)]) * 128).astype(np.int64)

    # sort by (core, dst-tile, src) -> ascending gather addresses per tile
    order = np.lexsort((src, tloc, core))
    sc, st = core[order], tloc[order]
    ssrc = src[order]
    sdst = dst[order]
    sattr = edge_attr.reshape(-1)[order]

    gid = sc * NT + st
    counts_flat = np.bincount(gid, minlength=NCORES * NT)
    offs = np.concatenate([[0], np.cumsum(counts_flat)])[:-1]
    rank = np.arange(E) - offs[gid]
    pos = starts[st] + rank

    esrc = np.zeros((NCORES, TE * 128), np.int32)
    attr_flat = np.zeros((NCORES, TE * 128), np.float32)
    dloc_flat = np.full((NCORES, TE * 128), -1, np.int64)
    esrc[sc, pos] = ssrc.astype(np.int32)
    attr_flat[sc, pos] = sattr
    dloc_flat[sc, pos] = (sdst % NPC) - st * 128

    esrc_T = np.ascontiguousarray(
        esrc.reshape(NCORES, TE, 128).transpose(0, 2, 1))
    eattr_T = np.ascontiguousarray(
        attr_flat.reshape(NCORES, TE, 128).transpose(0, 2, 1)).astype(NP_BF16)

    one8 = np.frombuffer(NP_FP8(1.0).tobytes(), np.uint8)[0]
    ind = np.zeros((NCORES, TE * 128, 128), np.uint8)
    cc, pp_ = np.nonzero(dloc_flat >= 0)
    ind[cc, pp_, dloc_flat[cc, pp_]] = one8
    ind = ind.reshape(NCORES, TE, 128, 128).transpose(0, 2, 1, 3)
    ind8 = np.ascontiguousarray(ind.reshape(NCORES, 128, TE * 128)).view(NP_FP8)

    return ET, esrc_T, eattr_T, ind8


def kernel(x, edge_index, edge_attr, batch, clinical,
           lin_src_w, lin_src_b, lin_dst_w, lin_dst_b,
           edge_w, edge_b, t,
           mlp_w1, mlp_b1, mlp_bn_g, mlp_bn_b, mlp_bn_m, mlp_bn_v,
           mlp_w2, mlp_b2, norm_g, norm_b, norm_m, norm_v,
           cls_w, cls_b):
    x = np.asarray(x, np.float32)
    edge_index = np.asarray(edge_index)
    edge_attr = np.asarray(edge_attr, np.float32)
    batch = np.asarray(batch)
    t = np.asarray(t, np.float32)

    ET, esrc_T, eattr_T, ind8 = _prep(edge_index, edge_attr)

    key = (tuple(int(v) for v in ET), t.tobytes())
    if key not in _cache:
        _cache.clear()
        _cache[key] = _build(ET, [float(v) for v in t])
    nc = _cache[key]

    # folded params (host, f32 math then bf16 cast)
    norm_g = np.asarray(norm_g, np.float32)
    norm_v = np.asarray(norm_v, np.float32)
    s_bn = norm_g / np.sqrt(norm_v + EPS_BN)
    b_bn = np.asarray(norm_b, np.float32) - np.asarray(norm_m, np.float32) * s_bn
    s1 = np.asarray(mlp_bn_g, np.float32) / np.sqrt(
        np.asarray(mlp_bn_v, np.float32) + EPS_BN)
    w1f = np.asarray(mlp_w1, np.float32) * s1[:, None, :]
    b1f = s1 * np.asarray(mlp_b1, np.float32) + (
        np.asarray(mlp_bn_b, np.float32) - np.asarray(mlp_bn_m, np.float32) * s1)
    ew = np.asarray(edge_w, np.float32)[:, 0, :]
    eb = np.asarray(edge_b, np.float32)
    lsb_fold = np.asarray(lin_src_b, np.float32) + eb[0]

    bcast = np.zeros((2 * L, P, P), np.float32)
    bcast[0] = np.tile(lsb_fold, (P, 1))
    for l in range(L):
        bcast[1 + l] = np.tile(ew[l], (P, 1))
    for l in range(1, L):
        bcast[4 + l] = np.tile(eb[l], (P, 1))

    xT = np.zeros((NCORES, C, NPC_PAD), NP_BF16)
    batch_T = np.full((NCORES, NPC_PAD), -1, np.int32)
    for c in range(NCORES):
        xT[c, :, :NPC] = x[c * NPC : (c + 1) * NPC].T.astype(NP_BF16)
        batch_T[c, :NPC] = batch[c * NPC : (c + 1) * NPC]
    batch_T = np.ascontiguousarray(
        batch_T.reshape(NCORES, NT, 128).transpose(0, 2, 1))

    shared = dict(
        bcast=bcast.astype(NP_BF16),
        lsw=np.asarray(lin_src_w, np.float32).astype(NP_BF16),
        ldw=np.asarray(lin_dst_w, np.float32).astype(NP_BF16),
        ldb=np.asarray(lin_dst_b, np.float32),
        w1f=np.ascontiguousarray(w1f.astype(NP_BF16)),
        b1f=np.ascontiguousarray(b1f),
        w2=np.ascontiguousarray(np.asarray(mlp_w2, np.float32).astype(NP_BF16)),
        b2=np.ascontiguousarray(np.asarray(mlp_b2, np.float32)),
        bns=np.ascontiguousarray(s_bn), bnb=np.ascontiguousarray(b_bn),
    )
    in_maps = [
        dict(shared, xT=np.ascontiguousarray(xT[c]), esrc=esrc_T[c],
             eattr=eattr_T[c], ind8=ind8[c], batch=batch_T[c])
        for c in range(NCORES)
    ]

    res = run_bass_kernel_spmd(nc, in_maps, core_ids=list(range(NCORES)))
    kernel.last = (nc, in_maps)

    pooled = np.zeros((G, H), np.float64)
    for c in range(NCORES):
        pooled += res.results[c]["pooled"].astype(np.float64)
    cnt = np.bincount(np.asarray(batch), minlength=G).astype(np.float64)
    pooled = (pooled / np.maximum(cnt, 1.0)[:, None]).astype(np.float32)
    z = np.concatenate([pooled, np.asarray(clinical, np.float32)], axis=1)
    return z @ np.asarray(cls_w, np.float32) + np.asarray(cls_b, np.float32)

